# revision 23
# baseline (speedup 1.0000x reference)
"""Trainium2 Bass kernel: 2-layer heterogeneous GCN + document aggregation,
8-core SPMD (dst-node sharding, AllGather of node tables, gather + one-hot-matmul spmm).

Self-contained: hardcodes all shapes. kernel(**inputs) -> (doc, doc_svd).

Tables are packed so each aggregation needs ONE row-gather per edge:
  l11_full   [8*S1P, 128]  = relu(x1@W1a+b1a)                 (e11 source)
  l12w_full  [8*S2P, 256]  = [l1_2 | wemb]                    (e22 source, col slice)
  l21c_full  [8*S1P, 256]  = [l2_1 | l1_1]                    (e01 source -> r0|r0s)
  l2all_full [8*S2P, 384]  = [l2_2 | l1_2 | wemb]             (e02 source -> r1/r1s parts)
"""

from contextlib import ExitStack

import numpy as np

import concourse.bacc as bacc
import concourse.mybir as mybir
from concourse.tile import TileContext
from concourse.bass_utils import run_bass_kernel_spmd

F32 = mybir.dt.float32
I16 = mybir.dt.int16
AF = mybir.ActivationFunctionType
ALU = mybir.AluOpType

P = 128
NC = 8
IDX_BATCH = 1024  # gather rows per dma_gather call (hard ucode limit: 2048 crashes)
EPS = 1e-9
KNOCKOUT = set()  # timing experiments: subsets of {"gather", "mm", "dve", "ag"}

# problem sizes
N0, N1, N2 = 10000, 50000, 30000
D, O, DW = 256, 128, 128

S0, S0P = N0 // NC, 1280   # doc shard: 1250 real rows, 10 tiles
S1, S1P = N1 // NC, 6272   # type-1 node shard: 6250 real, 49 tiles
S2, S2P = N2 // NC, 3840   # type-2 node shard: 3750 real, 30 tiles
T0, T1, T2 = S0P // P, S1P // P, S2P // P
HALF1 = 4 * S1P            # 25088: int16-safe half boundary of padded N1 tables


# ---------------------------------------------------------------------------
# Host-side edge preprocessing
# ---------------------------------------------------------------------------

class EdgePlan:
    """Static (core-independent) schedule + per-core data arrays for one edge set."""

    def __init__(self, src, dst, w, sd_real, sd_pad, ss_real, ss_pad, half_bound):
        src = np.asarray(src).astype(np.int64)
        dst = np.asarray(dst).astype(np.int64)
        w = np.asarray(w, np.float32)
        E = len(src)
        n_tiles = sd_pad // P
        core = dst // sd_real
        dloc = dst - core * sd_real
        tile_id = dloc // P
        dst_rel = (dloc % P).astype(np.float32)
        spad = (src // ss_real) * ss_pad + (src % ss_real)
        if half_bound:
            half = (spad >= half_bound).astype(np.int64)
            idx_rel = (spad - half * half_bound).astype(np.int64)
            n_halves = 2
        else:
            half = np.zeros(E, np.int64)
            idx_rel = spad
            n_halves = 1
        assert idx_rel.max() < 32768
        key = (core * n_halves + half) * n_tiles + tile_id
        counts = np.bincount(key, minlength=NC * n_halves * n_tiles).reshape(
            NC, n_halves, n_tiles
        )
        n_chunks = (-(-counts // P)).max(axis=0)  # [n_halves, n_tiles] cross-core max
        flat = n_chunks.reshape(-1)
        coff = np.concatenate([[0], np.cumsum(flat)[:-1]]).reshape(n_halves, n_tiles)
        total_chunks = int(flat.sum())

        order = np.argsort(key, kind="stable")
        key_s = key[order]
        grp_first = np.searchsorted(key_s, np.arange(NC * n_halves * n_tiles), "left")
        pos_in_grp = np.arange(E) - grp_first[key_s]
        spos = coff[half[order], tile_id[order]] * P + pos_in_grp

        idx_flat = np.zeros((NC, total_chunks * P), np.int16)
        w_flat = np.zeros((NC, total_chunks * P), np.float32)
        rel_flat = np.zeros((NC, total_chunks * P), np.float32)
        c_s = core[order]
        idx_flat[c_s, spos] = idx_rel[order].astype(np.int16)
        w_flat[c_s, spos] = w[order]
        rel_flat[c_s, spos] = dst_rel[order]

        self.n_halves = n_halves
        self.n_tiles = n_tiles
        self.n_chunks = n_chunks
        self.total_chunks = total_chunks
        self.half_nchunks = [int(n_chunks[h].sum()) for h in range(n_halves)]
        self.half_chunk_base = np.concatenate([[0], np.cumsum(self.half_nchunks)])
        self.sched = []  # per half: list of (tile, first, last) per chunk
        for h in range(n_halves):
            s = []
            for t in range(n_tiles):
                nct = int(n_chunks[h][t])
                for k in range(nct):
                    s.append((t, k == 0, k == nct - 1))
            self.sched.append(s)
        self.idx_wrapped = []  # [core][half] -> [128, Lh//16] int16
        self.w_wrapped = np.zeros((NC, P, total_chunks), np.float32)
        self.rel_wrapped = np.zeros((NC, P, total_chunks), np.float32)
        for c in range(NC):
            per_half = []
            for h in range(n_halves):
                lo = int(self.half_chunk_base[h]) * P
                hi = int(self.half_chunk_base[h + 1]) * P
                seg = idx_flat[c, lo:hi]
                wr = (np.tile(seg.reshape(-1, 16).T, (8, 1)) if hi > lo
                      else np.zeros((P, 0), np.int16))
                per_half.append(np.ascontiguousarray(wr))
            self.idx_wrapped.append(per_half)
            self.w_wrapped[c] = w_flat[c].reshape(total_chunks, P).T
            self.rel_wrapped[c] = rel_flat[c].reshape(total_chunks, P).T


# ---------------------------------------------------------------------------
# Device program pieces
# ---------------------------------------------------------------------------

def _dense_layer(nc, sb, psp, xT_in, n_node_tiles, wk0, wk1, b_row, ones, outb):
    """outb[:, m, :] = relu(xT[:, m-tile].T @ W + b) over both K halves."""
    xa0 = sb.tile([P, n_node_tiles * P], F32, tag="xa0")
    xa1 = sb.tile([P, n_node_tiles * P], F32, tag="xa1")
    nc.sync.dma_start(xa0[:], xT_in[0:P, :])
    nc.sync.dma_start(xa1[:], xT_in[P : 2 * P, :])
    for m in range(n_node_tiles):
        ps = psp.tile([P, P], F32, tag="psA")
        sl = slice(m * P, (m + 1) * P)
        nc.tensor.matmul(ps[:], lhsT=xa0[:, sl], rhs=wk0[:], start=True, stop=False)
        nc.tensor.matmul(ps[:], lhsT=xa1[:, sl], rhs=wk1[:], start=False, stop=False)
        nc.tensor.matmul(ps[:], lhsT=ones[:1, :], rhs=b_row[:1, :], start=False, stop=True)
        nc.scalar.activation(outb[:, m, :], ps[:], AF.Relu)


def _spmm(nc, gpool, spool, psp, plan, tables, idx_tiles, w_col, rel_col, iota,
          accN, accT, width, gtag, pstag):
    """Shared spmm walker. Per 128-edge chunk builds S = onehot(dst_rel)*w, then:
      accT given (Option T, width==P): psum[feat, dst] += G_chunk.T @ S
      accN given (Option N): psum[dst, 0:width] += S.T @ G_chunk -> accN[:, t, :]
    Halves accumulate into SBUF acc via copy-then-add."""
    n_tiles = plan.n_tiles
    written = [False] * n_tiles
    gchunk = 0
    for h in range(plan.n_halves):
        nch_h = plan.half_nchunks[h]
        if nch_h == 0:
            continue
        Lh = nch_h * P
        sched = plan.sched[h]
        table_ap, step = tables[h]
        ci = 0
        psum = None
        for b0 in range(0, Lh, IDX_BATCH):
            nidx = min(IDX_BATCH, Lh - b0)
            nch = nidx // P
            gt = gpool.tile([P, IDX_BATCH // P, width], F32, tag=gtag)
            if "gather" not in KNOCKOUT:
                nc.gpsimd.dma_gather(
                    gt[:, :nch, :], table_ap,
                    idx_tiles[h][:, b0 // 16 : (b0 + nidx) // 16],
                    nidx, nidx, width, elem_step=step,
                )
            else:
                nc.vector.memset(gt[:, :1, :8], 0.0)
            for k in range(nch):
                t, first, last = sched[ci]
                if first:
                    psum = psp.tile([P, width], F32, tag=pstag)
                S = spool.tile([P, P], F32, tag="S")
                if "dve" not in KNOCKOUT:
                    nc.vector.tensor_scalar(
                        S[:], iota[:], rel_col[:, gchunk : gchunk + 1],
                        w_col[:, gchunk : gchunk + 1], ALU.is_equal, ALU.mult,
                    )
                else:
                    nc.vector.memset(S[:, :8], 0.0)
                if "mm" not in KNOCKOUT:
                    if accN is not None:
                        nc.tensor.matmul(psum[:], lhsT=S[:], rhs=gt[:, k, :],
                                         start=first, stop=last)
                    else:
                        nc.tensor.matmul(psum[:], lhsT=gt[:, k, :], rhs=S[:],
                                         start=first, stop=last)
                elif first:
                    nc.tensor.matmul(psum[:, 0:P], lhsT=iota[:1, :], rhs=iota[:1, :],
                                     start=True, stop=True)
                if last:
                    sl = (accN[:, t, :] if accN is not None
                          else accT[:, t * P : (t + 1) * P])
                    if not written[t]:
                        nc.scalar.activation(sl, psum[:], AF.Copy)
                        written[t] = True
                    else:
                        nc.vector.tensor_tensor(sl, psum[:], sl, ALU.add)
                ci += 1
                gchunk += 1
    for t in range(n_tiles):
        if not written[t]:
            sl = accN[:, t, :] if accN is not None else accT[:, t * P : (t + 1) * P]
            nc.vector.memset(sl, 0.0)


def _gcn_second(nc, psp, accT, w_t, b_row, ones, outc, n_tiles):
    """outc[:, t, 0:O] = relu(accT_t.T @ W + b)."""
    for t in range(n_tiles):
        ps = psp.tile([P, P], F32, tag="ps2")
        nc.tensor.matmul(ps[:], lhsT=accT[:, t * P : (t + 1) * P], rhs=w_t[:],
                         start=True, stop=False)
        nc.tensor.matmul(ps[:], lhsT=ones[:1, :], rhs=b_row[:1, :], start=False, stop=True)
        nc.scalar.activation(outc[:, t, 0:O], ps[:], AF.Relu)


def _edge_phase_loads(nc, sb, plan, idx_in, w_in, rel_in, tagp):
    idx_t = []
    for h in range(plan.n_halves):
        it = sb.tile(list(plan.idx_wrapped[0][h].shape), I16, tag=f"{tagp}i{h}")
        nc.sync.dma_start(it[:], idx_in[h][:])
        idx_t.append(it)
    wct = sb.tile([P, plan.total_chunks], F32, tag=f"{tagp}w")
    relt = sb.tile([P, plan.total_chunks], F32, tag=f"{tagp}r")
    nc.sync.dma_start(wct[:], w_in[:])
    nc.sync.dma_start(relt[:], rel_in[:])
    return idx_t, wct, relt


# ---------------------------------------------------------------------------
# Full program
# ---------------------------------------------------------------------------

def build_program(p11, p22, p01, p02, repeat=1):
    nc = bacc.Bacc("TRN2", num_devices=NC)

    x1T = nc.dram_tensor("x1T", [D, S1P], F32, kind="ExternalInput")
    x2T = nc.dram_tensor("x2T", [D, S2P], F32, kind="ExternalInput")
    wemb_sh = nc.dram_tensor("wemb_sh", [S2P, DW], F32, kind="ExternalInput")
    w1a = nc.dram_tensor("w1a", [D, O], F32, kind="ExternalInput")
    w1b = nc.dram_tensor("w1b", [O, O], F32, kind="ExternalInput")
    w2a = nc.dram_tensor("w2a", [D, O], F32, kind="ExternalInput")
    w2b = nc.dram_tensor("w2b", [O, O], F32, kind="ExternalInput")
    biases = nc.dram_tensor("biases", [4, O], F32, kind="ExternalInput")
    iota_in = nc.dram_tensor("iota", [P, P], F32, kind="ExternalInput")
    ones_in = nc.dram_tensor("ones", [1, P], F32, kind="ExternalInput")

    def edge_inputs(name, plan):
        idx = [
            nc.dram_tensor(f"{name}_idx{h}", list(plan.idx_wrapped[0][h].shape), I16,
                           kind="ExternalInput")
            for h in range(plan.n_halves)
        ]
        wv = nc.dram_tensor(f"{name}_w", [P, plan.total_chunks], F32, kind="ExternalInput")
        rel = nc.dram_tensor(f"{name}_rel", [P, plan.total_chunks], F32, kind="ExternalInput")
        return idx, wv, rel

    e11_in = edge_inputs("e11", p11)
    e22_in = edge_inputs("e22", p22)
    e01_in = edge_inputs("e01", p01)
    e02_in = edge_inputs("e02", p02)

    l11_loc = nc.dram_tensor("l11_loc", [S1P, O], F32)
    l12w_loc = nc.dram_tensor("l12w_loc", [S2P, 2 * O], F32)
    l21c_loc = nc.dram_tensor("l21c_loc", [S1P, 2 * O], F32)
    l2all_loc = nc.dram_tensor("l2all_loc", [S2P, 3 * O], F32)
    l11_full = nc.dram_tensor("l11_full", [NC * S1P, O], F32, addr_space="Shared")
    l12w_full = nc.dram_tensor("l12w_full", [NC * S2P, 2 * O], F32, addr_space="Shared")
    l21c_full = nc.dram_tensor("l21c_full", [NC * S1P, 2 * O], F32, addr_space="Shared")
    l2all_full = nc.dram_tensor("l2all_full", [NC * S2P, 3 * O], F32, addr_space="Shared")

    doc_loc = nc.dram_tensor("doc_local", [S0P, 2 * O + DW], F32, kind="ExternalOutput")
    docsvd_loc = nc.dram_tensor("docsvd_local", [S0P, 2 * O + DW], F32,
                                kind="ExternalOutput")

    rg = [list(range(NC))]

    def ag(inp, outp):
        if "ag" not in KNOCKOUT:
            nc.gpsimd.collective_compute(
                "AllGather", ALU.bypass, replica_groups=rg, ins=[inp[:]], outs=[outp[:]])

    def rearr(dram_ap):
        return dram_ap.rearrange("(t p) f -> p t f", p=P)

    with TileContext(nc) as tc:
        with tc.tile_pool(name="const", bufs=1) as cp:
            iota = cp.tile([P, P], F32)
            ones = cp.tile([1, P], F32)
            nc.sync.dma_start(iota[:], iota_in[:])
            nc.sync.dma_start(ones[:], ones_in[:])
            w1a0 = cp.tile([P, O], F32); nc.sync.dma_start(w1a0[:], w1a[0:P, :])
            w1a1 = cp.tile([P, O], F32); nc.sync.dma_start(w1a1[:], w1a[P:D, :])
            w2a0 = cp.tile([P, O], F32); nc.sync.dma_start(w2a0[:], w2a[0:P, :])
            w2a1 = cp.tile([P, O], F32); nc.sync.dma_start(w2a1[:], w2a[P:D, :])
            w1bt = cp.tile([O, O], F32); nc.sync.dma_start(w1bt[:], w1b[:])
            w2bt = cp.tile([O, O], F32); nc.sync.dma_start(w2bt[:], w2b[:])
            btiles = []
            for i in range(4):
                bt = cp.tile([1, O], F32, tag=f"b{i}")
                nc.sync.dma_start(bt[:], biases[i : i + 1, :])
                btiles.append(bt)
            b1a, b1b, b2a, b2b = (bt[:] for bt in btiles)

            def emit_body():
                with ExitStack() as ab:
                    psp = ab.enter_context(
                        tc.tile_pool(name="psum", bufs=2, space="PSUM"))
                    pspw = ab.enter_context(
                        tc.tile_pool(name="psumW", bufs=2, space="PSUM"))

                    # ---- phase A: identity GCN layers + wemb packing ----
                    with tc.tile_pool(name="phA", bufs=1) as sa:
                        outb1 = sa.tile([P, T1, O], F32)
                        _dense_layer(nc, sa, psp, x1T, T1, w1a0, w1a1, b1a, ones, outb1)
                        nc.sync.dma_start(rearr(l11_loc[:]), outb1[:])
                        outb2 = sa.tile([P, T2, O], F32)
                        _dense_layer(nc, sa, psp, x2T, T2, w2a0, w2a1, b2a, ones, outb2)
                        wt2 = sa.tile([P, T2, DW], F32)
                        nc.sync.dma_start(wt2[:], rearr(wemb_sh[:]))
                        nc.sync.dma_start(rearr(l12w_loc[:])[:, :, 0:O], outb2[:])
                        nc.sync.dma_start(rearr(l12w_loc[:])[:, :, O : 2 * O], wt2[:])

                    ag(l11_loc, l11_full)
                    ag(l12w_loc, l12w_full)

                    l11_halves = [(l11_full[0:HALF1, :], None),
                                  (l11_full[HALF1 : 2 * HALF1, :], None)]

                    # ---- phase B1: spmm(e11, l1_1) -> @W1b -> [l2_1 | l1_1] ----
                    with (
                        tc.tile_pool(name="phB1", bufs=1) as sb1,
                        tc.tile_pool(name="gpB1", bufs=3) as gp1,
                        tc.tile_pool(name="spB1", bufs=4) as sp1,
                    ):
                        idx_t, wct, relt = _edge_phase_loads(
                            nc, sb1, p11, e11_in[0], e11_in[1], e11_in[2], tagp="a")
                        accT = sb1.tile([P, S1P], F32, tag="accT")
                        _spmm(nc, gp1, sp1, psp, p11, l11_halves, idx_t, wct, relt,
                              iota, None, accT, P, "gT", "psT")
                        outc = sb1.tile([P, T1, 2 * O], F32, tag="outc")
                        _gcn_second(nc, pspw, accT, w1bt, b1b, ones, outc, T1)
                        nc.sync.dma_start(outc[:, :, O : 2 * O], rearr(l11_loc[:]))
                        nc.sync.dma_start(rearr(l21c_loc[:]), outc[:])

                    ag(l21c_loc, l21c_full)

                    # ---- phase B2: spmm(e22, l1_2) -> @W2b -> [l2_2 | l1_2 | wemb] ----
                    with (
                        tc.tile_pool(name="phB2", bufs=1) as sb2,
                        tc.tile_pool(name="gpB2", bufs=3) as gp2,
                        tc.tile_pool(name="spB2", bufs=4) as sp2,
                    ):
                        idx_t, wct, relt = _edge_phase_loads(
                            nc, sb2, p22, e22_in[0], e22_in[1], e22_in[2], tagp="b")
                        accT = sb2.tile([P, S2P], F32, tag="accT")
                        _spmm(nc, gp2, sp2, psp, p22, [(l12w_full[:, 0:O], 2 * O)],
                              idx_t, wct, relt, iota, None, accT, P, "gT", "psT")
                        outc2 = sb2.tile([P, T2, 3 * O], F32, tag="outc2")
                        _gcn_second(nc, pspw, accT, w2bt, b2b, ones, outc2, T2)
                        nc.sync.dma_start(outc2[:, :, O : 3 * O], rearr(l12w_loc[:]))
                        nc.sync.dma_start(rearr(l2all_loc[:]), outc2[:])

                    ag(l2all_loc, l2all_full)

                # ---- phase C: doc aggregation (A/B PSUM pools closed) ----
                with (
                    tc.tile_pool(name="phC", bufs=1) as sc,
                    tc.tile_pool(name="psC", bufs=2, space="PSUM") as psc,
                ):
                    acc01 = sc.tile([P, T0, 2 * O], F32, tag="acc01")  # [r0 | r0s]
                    acc02 = sc.tile([P, T0, 3 * O], F32, tag="acc02")  # [l22|l12|wemb]

                    with (
                        tc.tile_pool(name="gpC1", bufs=3) as gpc1,
                        tc.tile_pool(name="spC1", bufs=4) as spc1,
                    ):
                        idx_t, wct, relt = _edge_phase_loads(
                            nc, sc, p01, e01_in[0], e01_in[1], e01_in[2], tagp="c")
                        tables = [(l21c_full[0:HALF1, :], None),
                                  (l21c_full[HALF1 : 2 * HALF1, :], None)]
                        _spmm(nc, gpc1, spc1, psc, p01, tables, idx_t, wct, relt,
                              iota, acc01[:], None, 2 * O, "g01", "ps01")

                    with (
                        tc.tile_pool(name="gpC2", bufs=3) as gpc2,
                        tc.tile_pool(name="spC2", bufs=4) as spc2,
                    ):
                        idx_t, wct, relt = _edge_phase_loads(
                            nc, sc, p02, e02_in[0], e02_in[1], e02_in[2], tagp="d")
                        _spmm(nc, gpc2, spc2, psc, p02, [(l2all_full[:], None)],
                              idx_t, wct, relt, iota, acc02[:], None, 3 * O, "g02", "ps02")

                    docb = sc.tile([P, T0, 2 * O + DW], F32, tag="docb")
                    docsb = sc.tile([P, T0, 2 * O + DW], F32, tag="docsb")

                    def norm_scale(acc_slices, out_writes):
                        ss_total = None
                        for i, (s_ap, wdt) in enumerate(acc_slices):
                            sq = sc.tile([P, wdt], F32, tag=f"sq{i}")
                            ss = sc.tile([P, 1], F32, tag=f"ss{i}")
                            nc.scalar.activation(sq[:], s_ap, AF.Square, accum_out=ss[:])
                            if ss_total is None:
                                ss_total = ss
                            else:
                                nc.vector.tensor_tensor(
                                    ss_total[:], ss[:], ss_total[:], ALU.add)
                        nrm = sc.tile([P, 1], F32, tag="nrm")
                        nc.scalar.activation(nrm[:], ss_total[:], AF.Sqrt)
                        nc.vector.tensor_scalar_add(nrm[:], nrm[:], EPS)
                        rn = sc.tile([P, 1], F32, tag="rn")
                        nc.vector.reciprocal(rn[:], nrm[:])
                        for dst_ap, s_ap in out_writes:
                            nc.vector.tensor_scalar_mul(dst_ap, s_ap, rn[:])

                    for t in range(T0):
                        a01 = acc01[:, t, :]
                        a02 = acc02[:, t, :]
                        # doc = [norm(r0) | norm([l22|wemb])]
                        norm_scale([(a01[:, 0:O], O)],
                                   [(docb[:, t, 0:O], a01[:, 0:O])])
                        norm_scale(
                            [(a02[:, 0:O], O), (a02[:, 2 * O : 3 * O], O)],
                            [(docb[:, t, O : 2 * O], a02[:, 0:O]),
                             (docb[:, t, 2 * O : 3 * O], a02[:, 2 * O : 3 * O])])
                        # doc_svd = [norm(r0s) | norm([l12|wemb])]
                        norm_scale([(a01[:, O : 2 * O], O)],
                                   [(docsb[:, t, 0:O], a01[:, O : 2 * O])])
                        norm_scale([(a02[:, O : 3 * O], 2 * O)],
                                   [(docsb[:, t, O : 3 * O], a02[:, O : 3 * O])])
                    nc.sync.dma_start(rearr(doc_loc[:]), docb[:])
                    nc.sync.dma_start(rearr(docsvd_loc[:]), docsb[:])

            for _ in range(repeat):
                emit_body()

    nc.compile()
    return nc


# ---------------------------------------------------------------------------
# Host wrapper
# ---------------------------------------------------------------------------

_CACHE = {}


def _prep(inputs):
    x1 = np.asarray(inputs["x1"], np.float32)
    x2 = np.asarray(inputs["x2"], np.float32)
    wemb = np.asarray(inputs["word_emb"], np.float32)

    p11 = EdgePlan(inputs["e11_src"], inputs["e11_dst"], inputs["e11_w"],
                   S1, S1P, S1, S1P, HALF1)
    p22 = EdgePlan(inputs["e22_src"], inputs["e22_dst"], inputs["e22_w"],
                   S2, S2P, S2, S2P, None)
    p01 = EdgePlan(inputs["e01_src"], inputs["e01_dst"], inputs["e01_w"],
                   S0, S0P, S1, S1P, HALF1)
    p02 = EdgePlan(inputs["e02_src"], inputs["e02_dst"], inputs["e02_w"],
                   S0, S0P, S2, S2P, None)

    iota = np.tile(np.arange(P, dtype=np.float32), (P, 1))
    ones = np.ones((1, P), np.float32)
    biases = np.stack([
        np.asarray(inputs["b1a"], np.float32), np.asarray(inputs["b1b"], np.float32),
        np.asarray(inputs["b2a"], np.float32), np.asarray(inputs["b2b"], np.float32),
    ])

    in_maps = []
    for c in range(NC):
        x1T = np.zeros((D, S1P), np.float32)
        x1T[:, :S1] = x1[c * S1 : (c + 1) * S1].T
        x2T = np.zeros((D, S2P), np.float32)
        x2T[:, :S2] = x2[c * S2 : (c + 1) * S2].T
        wsh = np.zeros((S2P, DW), np.float32)
        wsh[:S2] = wemb[c * S2 : (c + 1) * S2]
        m = {
            "x1T": x1T, "x2T": x2T, "wemb_sh": wsh,
            "w1a": np.asarray(inputs["W1a"], np.float32),
            "w1b": np.asarray(inputs["W1b"], np.float32),
            "w2a": np.asarray(inputs["W2a"], np.float32),
            "w2b": np.asarray(inputs["W2b"], np.float32),
            "biases": biases, "iota": iota, "ones": ones,
        }
        for name, plan in (("e11", p11), ("e22", p22), ("e01", p01), ("e02", p02)):
            for h in range(plan.n_halves):
                m[f"{name}_idx{h}"] = plan.idx_wrapped[c][h]
            m[f"{name}_w"] = np.ascontiguousarray(plan.w_wrapped[c])
            m[f"{name}_rel"] = np.ascontiguousarray(plan.rel_wrapped[c])
        in_maps.append(m)
    return (p11, p22, p01, p02), in_maps


def get_compiled(inputs):
    plans, in_maps = _prep(inputs)
    key = tuple(p.total_chunks for p in plans) + tuple(
        tuple(p.n_chunks.reshape(-1).tolist()) for p in plans
    )
    if key not in _CACHE:
        _CACHE[key] = build_program(*plans)
    return _CACHE[key], in_maps


def kernel(**inputs):
    nc, in_maps = get_compiled(inputs)
    res = run_bass_kernel_spmd(nc, in_maps, core_ids=list(range(NC)), trace=False)
    doc = np.concatenate([res.results[c]["doc_local"][:S0] for c in range(NC)], axis=0)
    dsvd = np.concatenate([res.results[c]["docsvd_local"][:S0] for c in range(NC)],
                          axis=0)
    return (doc[:N0], dsvd[:N0])


# revision 26
# speedup vs baseline: 1096.9553x; 1096.9553x over previous
"""Trainium2 Bass kernel: 2-layer heterogeneous GCN + document aggregation,
8-core SPMD (dst-node sharding, AllGather of node tables, gather + one-hot-matmul spmm).

Self-contained: hardcodes all shapes. kernel(**inputs) -> (doc, doc_svd).

Tables are packed so each aggregation needs ONE row-gather per edge:
  l11_full   [8*S1P, 128]  = relu(x1@W1a+b1a)                 (e11 source)
  l12w_full  [8*S2P, 256]  = [l1_2 | wemb]                    (e22 source, col slice)
  l21c_full  [8*S1P, 256]  = [l2_1 | l1_1]                    (e01 source -> r0|r0s)
  l2all_full [8*S2P, 384]  = [l2_2 | l1_2 | wemb]             (e02 source -> r1/r1s parts)
"""

from contextlib import ExitStack

import numpy as np

import concourse.bacc as bacc
import concourse.mybir as mybir
from concourse.tile import TileContext
from concourse.bass_utils import run_bass_kernel_spmd

F32 = mybir.dt.float32
I16 = mybir.dt.int16
AF = mybir.ActivationFunctionType
ALU = mybir.AluOpType

P = 128
NC = 8
SINGLE_PACKET = True
DMA_SCRATCH = 16384  # SWDGE descriptor-ring carveout (64KB tested: no gain, keep default)
IDX_BATCH = 1024  # gather rows per dma_gather call (hard ucode limit: 2048 crashes)
EPS = 1e-9
KNOCKOUT = set()  # timing experiments: subsets of {"gather", "mm", "dve", "ag"}

# problem sizes
N0, N1, N2 = 10000, 50000, 30000
D, O, DW = 256, 128, 128

S0, S0P = N0 // NC, 1280   # doc shard: 1250 real rows, 10 tiles
S1, S1P = N1 // NC, 6272   # type-1 node shard: 6250 real, 49 tiles
S2, S2P = N2 // NC, 3840   # type-2 node shard: 3750 real, 30 tiles
T0, T1, T2 = S0P // P, S1P // P, S2P // P
HALF1 = 4 * S1P            # 25088: int16-safe half boundary of padded N1 tables


# ---------------------------------------------------------------------------
# Host-side edge preprocessing
# ---------------------------------------------------------------------------

class EdgePlan:
    """Static (core-independent) schedule + per-core data arrays for one edge set."""

    def __init__(self, src, dst, w, sd_real, sd_pad, ss_real, ss_pad, half_bound):
        src = np.asarray(src).astype(np.int64)
        dst = np.asarray(dst).astype(np.int64)
        w = np.asarray(w, np.float32)
        E = len(src)
        n_tiles = sd_pad // P
        core = dst // sd_real
        dloc = dst - core * sd_real
        tile_id = dloc // P
        dst_rel = (dloc % P).astype(np.float32)
        spad = (src // ss_real) * ss_pad + (src % ss_real)
        if half_bound:
            half = (spad >= half_bound).astype(np.int64)
            idx_rel = (spad - half * half_bound).astype(np.int64)
            n_halves = 2
        else:
            half = np.zeros(E, np.int64)
            idx_rel = spad
            n_halves = 1
        assert idx_rel.max() < 32768
        key = (core * n_halves + half) * n_tiles + tile_id
        counts = np.bincount(key, minlength=NC * n_halves * n_tiles).reshape(
            NC, n_halves, n_tiles
        )
        n_chunks = (-(-counts // P)).max(axis=0)  # [n_halves, n_tiles] cross-core max
        flat = n_chunks.reshape(-1)
        coff = np.concatenate([[0], np.cumsum(flat)[:-1]]).reshape(n_halves, n_tiles)
        total_chunks = int(flat.sum())

        order = np.argsort(key, kind="stable")
        key_s = key[order]
        grp_first = np.searchsorted(key_s, np.arange(NC * n_halves * n_tiles), "left")
        pos_in_grp = np.arange(E) - grp_first[key_s]
        spos = coff[half[order], tile_id[order]] * P + pos_in_grp

        idx_flat = np.zeros((NC, total_chunks * P), np.int16)
        w_flat = np.zeros((NC, total_chunks * P), np.float32)
        rel_flat = np.zeros((NC, total_chunks * P), np.float32)
        c_s = core[order]
        idx_flat[c_s, spos] = idx_rel[order].astype(np.int16)
        w_flat[c_s, spos] = w[order]
        rel_flat[c_s, spos] = dst_rel[order]

        self.n_halves = n_halves
        self.n_tiles = n_tiles
        self.n_chunks = n_chunks
        self.total_chunks = total_chunks
        self.half_nchunks = [int(n_chunks[h].sum()) for h in range(n_halves)]
        self.half_chunk_base = np.concatenate([[0], np.cumsum(self.half_nchunks)])
        self.sched = []  # per half: list of (tile, first, last) per chunk
        for h in range(n_halves):
            s = []
            for t in range(n_tiles):
                nct = int(n_chunks[h][t])
                for k in range(nct):
                    s.append((t, k == 0, k == nct - 1))
            self.sched.append(s)
        self.idx_wrapped = []  # [core][half] -> [128, Lh//16] int16
        self.w_wrapped = np.zeros((NC, P, total_chunks), np.float32)
        self.rel_wrapped = np.zeros((NC, P, total_chunks), np.float32)
        for c in range(NC):
            per_half = []
            for h in range(n_halves):
                lo = int(self.half_chunk_base[h]) * P
                hi = int(self.half_chunk_base[h + 1]) * P
                seg = idx_flat[c, lo:hi]
                wr = (np.tile(seg.reshape(-1, 16).T, (8, 1)) if hi > lo
                      else np.zeros((P, 0), np.int16))
                per_half.append(np.ascontiguousarray(wr))
            self.idx_wrapped.append(per_half)
            self.w_wrapped[c] = w_flat[c].reshape(total_chunks, P).T
            self.rel_wrapped[c] = rel_flat[c].reshape(total_chunks, P).T


# ---------------------------------------------------------------------------
# Device program pieces
# ---------------------------------------------------------------------------

def _dense_layer(nc, sb, psp, xT_in, n_node_tiles, wk0, wk1, b_row, ones, outb):
    """outb[:, m, :] = relu(xT[:, m-tile].T @ W + b) over both K halves."""
    xa0 = sb.tile([P, n_node_tiles * P], F32, tag="xa0")
    xa1 = sb.tile([P, n_node_tiles * P], F32, tag="xa1")
    nc.sync.dma_start(xa0[:], xT_in[0:P, :])
    nc.sync.dma_start(xa1[:], xT_in[P : 2 * P, :])
    for m in range(n_node_tiles):
        ps = psp.tile([P, P], F32, tag="psA")
        sl = slice(m * P, (m + 1) * P)
        nc.tensor.matmul(ps[:], lhsT=xa0[:, sl], rhs=wk0[:], start=True, stop=False)
        nc.tensor.matmul(ps[:], lhsT=xa1[:, sl], rhs=wk1[:], start=False, stop=False)
        nc.tensor.matmul(ps[:], lhsT=ones[:1, :], rhs=b_row[:1, :], start=False, stop=True)
        nc.scalar.activation(outb[:, m, :], ps[:], AF.Relu)


def _spmm(nc, gpool, spool, psp, plan, tables, idx_tiles, w_col, rel_col, iota,
          accN, accT, width, gtag, pstag):
    """Shared spmm walker. Per 128-edge chunk builds S = onehot(dst_rel)*w, then:
      accT given (Option T, width==P): psum[feat, dst] += G_chunk.T @ S
      accN given (Option N): psum[dst, 0:width] += S.T @ G_chunk -> accN[:, t, :]
    Halves accumulate into SBUF acc via copy-then-add."""
    n_tiles = plan.n_tiles
    written = [False] * n_tiles
    gchunk = 0
    for h in range(plan.n_halves):
        nch_h = plan.half_nchunks[h]
        if nch_h == 0:
            continue
        Lh = nch_h * P
        sched = plan.sched[h]
        table_ap, step = tables[h]
        ci = 0
        psum = None
        for b0 in range(0, Lh, IDX_BATCH):
            nidx = min(IDX_BATCH, Lh - b0)
            nch = nidx // P
            gt = gpool.tile([P, IDX_BATCH // P, width], F32, tag=gtag)
            if "gather" not in KNOCKOUT:
                nc.gpsimd.dma_gather(
                    gt[:, :nch, :], table_ap,
                    idx_tiles[h][:, b0 // 16 : (b0 + nidx) // 16],
                    nidx, nidx, width, elem_step=step, single_packet=SINGLE_PACKET,
                )
            else:
                nc.vector.memset(gt[:, :1, :8], 0.0)
            for k in range(nch):
                t, first, last = sched[ci]
                if first:
                    psum = psp.tile([P, width], F32, tag=pstag)
                S = spool.tile([P, P], F32, tag="S")
                if "dve" not in KNOCKOUT:
                    nc.vector.tensor_scalar(
                        S[:], iota[:], rel_col[:, gchunk : gchunk + 1],
                        w_col[:, gchunk : gchunk + 1], ALU.is_equal, ALU.mult,
                    )
                else:
                    nc.vector.memset(S[:, :8], 0.0)
                if "mm" not in KNOCKOUT:
                    if accN is not None:
                        nc.tensor.matmul(psum[:], lhsT=S[:], rhs=gt[:, k, :],
                                         start=first, stop=last)
                    else:
                        nc.tensor.matmul(psum[:], lhsT=gt[:, k, :], rhs=S[:],
                                         start=first, stop=last)
                elif first:
                    nc.tensor.matmul(psum[:, 0:P], lhsT=iota[:1, :], rhs=iota[:1, :],
                                     start=True, stop=True)
                if last:
                    sl = (accN[:, t, :] if accN is not None
                          else accT[:, t * P : (t + 1) * P])
                    if not written[t]:
                        nc.scalar.activation(sl, psum[:], AF.Copy)
                        written[t] = True
                    else:
                        nc.vector.tensor_tensor(sl, psum[:], sl, ALU.add)
                ci += 1
                gchunk += 1
    for t in range(n_tiles):
        if not written[t]:
            sl = accN[:, t, :] if accN is not None else accT[:, t * P : (t + 1) * P]
            nc.vector.memset(sl, 0.0)


def _gcn_second(nc, psp, accT, w_t, b_row, ones, outc, n_tiles):
    """outc[:, t, 0:O] = relu(accT_t.T @ W + b)."""
    for t in range(n_tiles):
        ps = psp.tile([P, P], F32, tag="ps2")
        nc.tensor.matmul(ps[:], lhsT=accT[:, t * P : (t + 1) * P], rhs=w_t[:],
                         start=True, stop=False)
        nc.tensor.matmul(ps[:], lhsT=ones[:1, :], rhs=b_row[:1, :], start=False, stop=True)
        nc.scalar.activation(outc[:, t, 0:O], ps[:], AF.Relu)


def _edge_phase_loads(nc, sb, plan, idx_in, w_in, rel_in, tagp):
    idx_t = []
    for h in range(plan.n_halves):
        it = sb.tile(list(plan.idx_wrapped[0][h].shape), I16, tag=f"{tagp}i{h}")
        nc.sync.dma_start(it[:], idx_in[h][:])
        idx_t.append(it)
    wct = sb.tile([P, plan.total_chunks], F32, tag=f"{tagp}w")
    relt = sb.tile([P, plan.total_chunks], F32, tag=f"{tagp}r")
    nc.sync.dma_start(wct[:], w_in[:])
    nc.sync.dma_start(relt[:], rel_in[:])
    return idx_t, wct, relt


# ---------------------------------------------------------------------------
# Full program
# ---------------------------------------------------------------------------

def build_program(p11, p22, p01, p02, repeat=1):
    nc = bacc.Bacc("TRN2", num_devices=NC, dynamic_dma_scratch_size=DMA_SCRATCH)

    x1T = nc.dram_tensor("x1T", [D, S1P], F32, kind="ExternalInput")
    x2T = nc.dram_tensor("x2T", [D, S2P], F32, kind="ExternalInput")
    wemb_sh = nc.dram_tensor("wemb_sh", [S2P, DW], F32, kind="ExternalInput")
    w1a = nc.dram_tensor("w1a", [D, O], F32, kind="ExternalInput")
    w1b = nc.dram_tensor("w1b", [O, O], F32, kind="ExternalInput")
    w2a = nc.dram_tensor("w2a", [D, O], F32, kind="ExternalInput")
    w2b = nc.dram_tensor("w2b", [O, O], F32, kind="ExternalInput")
    biases = nc.dram_tensor("biases", [4, O], F32, kind="ExternalInput")
    iota_in = nc.dram_tensor("iota", [P, P], F32, kind="ExternalInput")
    ones_in = nc.dram_tensor("ones", [1, P], F32, kind="ExternalInput")

    def edge_inputs(name, plan):
        idx = [
            nc.dram_tensor(f"{name}_idx{h}", list(plan.idx_wrapped[0][h].shape), I16,
                           kind="ExternalInput")
            for h in range(plan.n_halves)
        ]
        wv = nc.dram_tensor(f"{name}_w", [P, plan.total_chunks], F32, kind="ExternalInput")
        rel = nc.dram_tensor(f"{name}_rel", [P, plan.total_chunks], F32, kind="ExternalInput")
        return idx, wv, rel

    e11_in = edge_inputs("e11", p11)
    e22_in = edge_inputs("e22", p22)
    e01_in = edge_inputs("e01", p01)
    e02_in = edge_inputs("e02", p02)

    l11_loc = nc.dram_tensor("l11_loc", [S1P, O], F32)
    l12w_loc = nc.dram_tensor("l12w_loc", [S2P, 2 * O], F32)
    l21c_loc = nc.dram_tensor("l21c_loc", [S1P, 2 * O], F32)
    l2all_loc = nc.dram_tensor("l2all_loc", [S2P, 3 * O], F32)
    l11_full = nc.dram_tensor("l11_full", [NC * S1P, O], F32, addr_space="Shared")
    l12w_full = nc.dram_tensor("l12w_full", [NC * S2P, 2 * O], F32, addr_space="Shared")
    l21c_full = nc.dram_tensor("l21c_full", [NC * S1P, 2 * O], F32, addr_space="Shared")
    l2all_full = nc.dram_tensor("l2all_full", [NC * S2P, 3 * O], F32, addr_space="Shared")

    doc_loc = nc.dram_tensor("doc_local", [S0P, 2 * O + DW], F32, kind="ExternalOutput")
    docsvd_loc = nc.dram_tensor("docsvd_local", [S0P, 2 * O + DW], F32,
                                kind="ExternalOutput")

    rg = [list(range(NC))]

    def ag(inp, outp):
        if "ag" not in KNOCKOUT:
            nc.gpsimd.collective_compute(
                "AllGather", ALU.bypass, replica_groups=rg, ins=[inp[:]], outs=[outp[:]])

    def rearr(dram_ap):
        return dram_ap.rearrange("(t p) f -> p t f", p=P)

    with TileContext(nc) as tc:
        with tc.tile_pool(name="const", bufs=1) as cp:
            iota = cp.tile([P, P], F32)
            ones = cp.tile([1, P], F32)
            nc.sync.dma_start(iota[:], iota_in[:])
            nc.sync.dma_start(ones[:], ones_in[:])
            w1a0 = cp.tile([P, O], F32); nc.sync.dma_start(w1a0[:], w1a[0:P, :])
            w1a1 = cp.tile([P, O], F32); nc.sync.dma_start(w1a1[:], w1a[P:D, :])
            w2a0 = cp.tile([P, O], F32); nc.sync.dma_start(w2a0[:], w2a[0:P, :])
            w2a1 = cp.tile([P, O], F32); nc.sync.dma_start(w2a1[:], w2a[P:D, :])
            w1bt = cp.tile([O, O], F32); nc.sync.dma_start(w1bt[:], w1b[:])
            w2bt = cp.tile([O, O], F32); nc.sync.dma_start(w2bt[:], w2b[:])
            btiles = []
            for i in range(4):
                bt = cp.tile([1, O], F32, tag=f"b{i}")
                nc.sync.dma_start(bt[:], biases[i : i + 1, :])
                btiles.append(bt)
            b1a, b1b, b2a, b2b = (bt[:] for bt in btiles)

            def emit_body():
                with ExitStack() as ab:
                    psp = ab.enter_context(
                        tc.tile_pool(name="psum", bufs=2, space="PSUM"))
                    pspw = ab.enter_context(
                        tc.tile_pool(name="psumW", bufs=2, space="PSUM"))

                    # ---- phase A: identity GCN layers + wemb packing ----
                    with tc.tile_pool(name="phA", bufs=1) as sa:
                        outb1 = sa.tile([P, T1, O], F32)
                        _dense_layer(nc, sa, psp, x1T, T1, w1a0, w1a1, b1a, ones, outb1)
                        nc.sync.dma_start(rearr(l11_loc[:]), outb1[:])
                        outb2 = sa.tile([P, T2, O], F32)
                        _dense_layer(nc, sa, psp, x2T, T2, w2a0, w2a1, b2a, ones, outb2)
                        wt2 = sa.tile([P, T2, DW], F32)
                        nc.sync.dma_start(wt2[:], rearr(wemb_sh[:]))
                        nc.sync.dma_start(rearr(l12w_loc[:])[:, :, 0:O], outb2[:])
                        nc.sync.dma_start(rearr(l12w_loc[:])[:, :, O : 2 * O], wt2[:])

                    ag(l11_loc, l11_full)
                    ag(l12w_loc, l12w_full)

                    l11_halves = [(l11_full[0:HALF1, :], None),
                                  (l11_full[HALF1 : 2 * HALF1, :], None)]

                    # ---- phase B1: spmm(e11, l1_1) -> @W1b -> [l2_1 | l1_1] ----
                    with (
                        tc.tile_pool(name="phB1", bufs=1) as sb1,
                        tc.tile_pool(name="gpB1", bufs=3) as gp1,
                        tc.tile_pool(name="spB1", bufs=4) as sp1,
                    ):
                        idx_t, wct, relt = _edge_phase_loads(
                            nc, sb1, p11, e11_in[0], e11_in[1], e11_in[2], tagp="a")
                        accT = sb1.tile([P, S1P], F32, tag="accT")
                        _spmm(nc, gp1, sp1, psp, p11, l11_halves, idx_t, wct, relt,
                              iota, None, accT, P, "gT", "psT")
                        outc = sb1.tile([P, T1, 2 * O], F32, tag="outc")
                        _gcn_second(nc, pspw, accT, w1bt, b1b, ones, outc, T1)
                        nc.sync.dma_start(outc[:, :, O : 2 * O], rearr(l11_loc[:]))
                        nc.sync.dma_start(rearr(l21c_loc[:]), outc[:])

                    ag(l21c_loc, l21c_full)

                    # ---- phase B2: spmm(e22, l1_2) -> @W2b -> [l2_2 | l1_2 | wemb] ----
                    with (
                        tc.tile_pool(name="phB2", bufs=1) as sb2,
                        tc.tile_pool(name="gpB2", bufs=3) as gp2,
                        tc.tile_pool(name="spB2", bufs=4) as sp2,
                    ):
                        idx_t, wct, relt = _edge_phase_loads(
                            nc, sb2, p22, e22_in[0], e22_in[1], e22_in[2], tagp="b")
                        accT = sb2.tile([P, S2P], F32, tag="accT")
                        _spmm(nc, gp2, sp2, psp, p22, [(l12w_full[:, 0:O], 2 * O)],
                              idx_t, wct, relt, iota, None, accT, P, "gT", "psT")
                        outc2 = sb2.tile([P, T2, 3 * O], F32, tag="outc2")
                        _gcn_second(nc, pspw, accT, w2bt, b2b, ones, outc2, T2)
                        nc.sync.dma_start(outc2[:, :, O : 3 * O], rearr(l12w_loc[:]))
                        nc.sync.dma_start(rearr(l2all_loc[:]), outc2[:])

                    ag(l2all_loc, l2all_full)

                # ---- phase C: doc aggregation (A/B PSUM pools closed) ----
                with (
                    tc.tile_pool(name="phC", bufs=1) as sc,
                    tc.tile_pool(name="psC", bufs=2, space="PSUM") as psc,
                ):
                    acc01 = sc.tile([P, T0, 2 * O], F32, tag="acc01")  # [r0 | r0s]
                    acc02 = sc.tile([P, T0, 3 * O], F32, tag="acc02")  # [l22|l12|wemb]

                    with (
                        tc.tile_pool(name="gpC1", bufs=3) as gpc1,
                        tc.tile_pool(name="spC1", bufs=4) as spc1,
                    ):
                        idx_t, wct, relt = _edge_phase_loads(
                            nc, sc, p01, e01_in[0], e01_in[1], e01_in[2], tagp="c")
                        tables = [(l21c_full[0:HALF1, :], None),
                                  (l21c_full[HALF1 : 2 * HALF1, :], None)]
                        _spmm(nc, gpc1, spc1, psc, p01, tables, idx_t, wct, relt,
                              iota, acc01[:], None, 2 * O, "g01", "ps01")

                    with (
                        tc.tile_pool(name="gpC2", bufs=3) as gpc2,
                        tc.tile_pool(name="spC2", bufs=4) as spc2,
                    ):
                        idx_t, wct, relt = _edge_phase_loads(
                            nc, sc, p02, e02_in[0], e02_in[1], e02_in[2], tagp="d")
                        _spmm(nc, gpc2, spc2, psc, p02, [(l2all_full[:], None)],
                              idx_t, wct, relt, iota, acc02[:], None, 3 * O, "g02", "ps02")

                    docb = sc.tile([P, T0, 2 * O + DW], F32, tag="docb")
                    docsb = sc.tile([P, T0, 2 * O + DW], F32, tag="docsb")

                    def norm_scale(acc_slices, out_writes):
                        ss_total = None
                        for i, (s_ap, wdt) in enumerate(acc_slices):
                            sq = sc.tile([P, wdt], F32, tag=f"sq{i}")
                            ss = sc.tile([P, 1], F32, tag=f"ss{i}")
                            nc.scalar.activation(sq[:], s_ap, AF.Square, accum_out=ss[:])
                            if ss_total is None:
                                ss_total = ss
                            else:
                                nc.vector.tensor_tensor(
                                    ss_total[:], ss[:], ss_total[:], ALU.add)
                        nrm = sc.tile([P, 1], F32, tag="nrm")
                        nc.scalar.activation(nrm[:], ss_total[:], AF.Sqrt)
                        nc.vector.tensor_scalar_add(nrm[:], nrm[:], EPS)
                        rn = sc.tile([P, 1], F32, tag="rn")
                        nc.vector.reciprocal(rn[:], nrm[:])
                        for dst_ap, s_ap in out_writes:
                            nc.vector.tensor_scalar_mul(dst_ap, s_ap, rn[:])

                    for t in range(T0):
                        a01 = acc01[:, t, :]
                        a02 = acc02[:, t, :]
                        # doc = [norm(r0) | norm([l22|wemb])]
                        norm_scale([(a01[:, 0:O], O)],
                                   [(docb[:, t, 0:O], a01[:, 0:O])])
                        norm_scale(
                            [(a02[:, 0:O], O), (a02[:, 2 * O : 3 * O], O)],
                            [(docb[:, t, O : 2 * O], a02[:, 0:O]),
                             (docb[:, t, 2 * O : 3 * O], a02[:, 2 * O : 3 * O])])
                        # doc_svd = [norm(r0s) | norm([l12|wemb])]
                        norm_scale([(a01[:, O : 2 * O], O)],
                                   [(docsb[:, t, 0:O], a01[:, O : 2 * O])])
                        norm_scale([(a02[:, O : 3 * O], 2 * O)],
                                   [(docsb[:, t, O : 3 * O], a02[:, O : 3 * O])])
                    nc.sync.dma_start(rearr(doc_loc[:]), docb[:])
                    nc.sync.dma_start(rearr(docsvd_loc[:]), docsb[:])

            for _ in range(repeat):
                emit_body()

    nc.compile()
    return nc


# ---------------------------------------------------------------------------
# Host wrapper
# ---------------------------------------------------------------------------

_CACHE = {}


def _prep(inputs):
    x1 = np.asarray(inputs["x1"], np.float32)
    x2 = np.asarray(inputs["x2"], np.float32)
    wemb = np.asarray(inputs["word_emb"], np.float32)

    p11 = EdgePlan(inputs["e11_src"], inputs["e11_dst"], inputs["e11_w"],
                   S1, S1P, S1, S1P, HALF1)
    p22 = EdgePlan(inputs["e22_src"], inputs["e22_dst"], inputs["e22_w"],
                   S2, S2P, S2, S2P, None)
    p01 = EdgePlan(inputs["e01_src"], inputs["e01_dst"], inputs["e01_w"],
                   S0, S0P, S1, S1P, HALF1)
    p02 = EdgePlan(inputs["e02_src"], inputs["e02_dst"], inputs["e02_w"],
                   S0, S0P, S2, S2P, None)

    iota = np.tile(np.arange(P, dtype=np.float32), (P, 1))
    ones = np.ones((1, P), np.float32)
    biases = np.stack([
        np.asarray(inputs["b1a"], np.float32), np.asarray(inputs["b1b"], np.float32),
        np.asarray(inputs["b2a"], np.float32), np.asarray(inputs["b2b"], np.float32),
    ])

    in_maps = []
    for c in range(NC):
        x1T = np.zeros((D, S1P), np.float32)
        x1T[:, :S1] = x1[c * S1 : (c + 1) * S1].T
        x2T = np.zeros((D, S2P), np.float32)
        x2T[:, :S2] = x2[c * S2 : (c + 1) * S2].T
        wsh = np.zeros((S2P, DW), np.float32)
        wsh[:S2] = wemb[c * S2 : (c + 1) * S2]
        m = {
            "x1T": x1T, "x2T": x2T, "wemb_sh": wsh,
            "w1a": np.asarray(inputs["W1a"], np.float32),
            "w1b": np.asarray(inputs["W1b"], np.float32),
            "w2a": np.asarray(inputs["W2a"], np.float32),
            "w2b": np.asarray(inputs["W2b"], np.float32),
            "biases": biases, "iota": iota, "ones": ones,
        }
        for name, plan in (("e11", p11), ("e22", p22), ("e01", p01), ("e02", p02)):
            for h in range(plan.n_halves):
                m[f"{name}_idx{h}"] = plan.idx_wrapped[c][h]
            m[f"{name}_w"] = np.ascontiguousarray(plan.w_wrapped[c])
            m[f"{name}_rel"] = np.ascontiguousarray(plan.rel_wrapped[c])
        in_maps.append(m)
    return (p11, p22, p01, p02), in_maps


def get_compiled(inputs):
    plans, in_maps = _prep(inputs)
    key = tuple(p.total_chunks for p in plans) + tuple(
        tuple(p.n_chunks.reshape(-1).tolist()) for p in plans
    )
    if key not in _CACHE:
        _CACHE[key] = build_program(*plans)
    return _CACHE[key], in_maps


def kernel(**inputs):
    nc, in_maps = get_compiled(inputs)
    res = run_bass_kernel_spmd(nc, in_maps, core_ids=list(range(NC)), trace=False)
    doc = np.concatenate([res.results[c]["doc_local"][:S0] for c in range(NC)], axis=0)
    dsvd = np.concatenate([res.results[c]["docsvd_local"][:S0] for c in range(NC)],
                          axis=0)
    return (doc[:N0], dsvd[:N0])


# revision 27
# speedup vs baseline: 3563.9430x; 3.2489x over previous
"""Trainium2 Bass kernel: 2-layer heterogeneous GCN + document aggregation,
8-core SPMD (dst-node sharding, AllGather of node tables, gather + one-hot-matmul spmm).

Self-contained: hardcodes all shapes. kernel(**inputs) -> (doc, doc_svd).

Tables are packed so each aggregation needs ONE row-gather per edge:
  l11_full   [8*S1P, 128]  = relu(x1@W1a+b1a)                 (e11 source)
  l12w_full  [8*S2P, 256]  = [l1_2 | wemb]                    (e22 source, col slice)
  l21c_full  [8*S1P, 256]  = [l2_1 | l1_1]                    (e01 source -> r0|r0s)
  l2all_full [8*S2P, 384]  = [l2_2 | l1_2 | wemb]             (e02 source -> r1/r1s parts)
"""

from contextlib import ExitStack

import numpy as np

import concourse.bacc as bacc
import concourse.mybir as mybir
from concourse.tile import TileContext
from concourse.bass_utils import run_bass_kernel_spmd

F32 = mybir.dt.float32
I16 = mybir.dt.int16
AF = mybir.ActivationFunctionType
ALU = mybir.AluOpType

P = 128
NC = 8
SINGLE_PACKET = True
GP_BUFS = 3   # in-flight gather batches per pool
SP_BUFS = 4   # one-hot S tiles in flight
DMA_SCRATCH = 16384  # SWDGE descriptor-ring carveout (64KB tested: no gain, keep default)
IDX_BATCH = 1024  # gather rows per dma_gather call (hard ucode limit: 2048 crashes)
EPS = 1e-9
KNOCKOUT = set()  # timing experiments: subsets of {"gather", "mm", "dve", "ag"}

# problem sizes
N0, N1, N2 = 10000, 50000, 30000
D, O, DW = 256, 128, 128

S0, S0P = N0 // NC, 1280   # doc shard: 1250 real rows, 10 tiles
S1, S1P = N1 // NC, 6272   # type-1 node shard: 6250 real, 49 tiles
S2, S2P = N2 // NC, 3840   # type-2 node shard: 3750 real, 30 tiles
T0, T1, T2 = S0P // P, S1P // P, S2P // P
HALF1 = 4 * S1P            # 25088: int16-safe half boundary of padded N1 tables


# ---------------------------------------------------------------------------
# Host-side edge preprocessing
# ---------------------------------------------------------------------------

class EdgePlan:
    """Static (core-independent) schedule + per-core data arrays for one edge set."""

    def __init__(self, src, dst, w, sd_real, sd_pad, ss_real, ss_pad, half_bound):
        src = np.asarray(src).astype(np.int64)
        dst = np.asarray(dst).astype(np.int64)
        w = np.asarray(w, np.float32)
        E = len(src)
        n_tiles = sd_pad // P
        core = dst // sd_real
        dloc = dst - core * sd_real
        tile_id = dloc // P
        dst_rel = (dloc % P).astype(np.float32)
        spad = (src // ss_real) * ss_pad + (src % ss_real)
        if half_bound:
            half = (spad >= half_bound).astype(np.int64)
            idx_rel = (spad - half * half_bound).astype(np.int64)
            n_halves = 2
        else:
            half = np.zeros(E, np.int64)
            idx_rel = spad
            n_halves = 1
        assert idx_rel.max() < 32768
        key = (core * n_halves + half) * n_tiles + tile_id
        counts = np.bincount(key, minlength=NC * n_halves * n_tiles).reshape(
            NC, n_halves, n_tiles
        )
        n_chunks = (-(-counts // P)).max(axis=0)  # [n_halves, n_tiles] cross-core max
        flat = n_chunks.reshape(-1)
        coff = np.concatenate([[0], np.cumsum(flat)[:-1]]).reshape(n_halves, n_tiles)
        total_chunks = int(flat.sum())

        order = np.argsort(key, kind="stable")
        key_s = key[order]
        grp_first = np.searchsorted(key_s, np.arange(NC * n_halves * n_tiles), "left")
        pos_in_grp = np.arange(E) - grp_first[key_s]
        spos = coff[half[order], tile_id[order]] * P + pos_in_grp

        idx_flat = np.zeros((NC, total_chunks * P), np.int16)
        w_flat = np.zeros((NC, total_chunks * P), np.float32)
        rel_flat = np.zeros((NC, total_chunks * P), np.float32)
        c_s = core[order]
        idx_flat[c_s, spos] = idx_rel[order].astype(np.int16)
        w_flat[c_s, spos] = w[order]
        rel_flat[c_s, spos] = dst_rel[order]

        self.n_halves = n_halves
        self.n_tiles = n_tiles
        self.n_chunks = n_chunks
        self.total_chunks = total_chunks
        self.half_nchunks = [int(n_chunks[h].sum()) for h in range(n_halves)]
        self.half_chunk_base = np.concatenate([[0], np.cumsum(self.half_nchunks)])
        self.sched = []  # per half: list of (tile, first, last) per chunk
        for h in range(n_halves):
            s = []
            for t in range(n_tiles):
                nct = int(n_chunks[h][t])
                for k in range(nct):
                    s.append((t, k == 0, k == nct - 1))
            self.sched.append(s)
        self.idx_wrapped = []  # [core][half] -> [128, Lh//16] int16
        self.w_wrapped = np.zeros((NC, P, total_chunks), np.float32)
        self.rel_wrapped = np.zeros((NC, P, total_chunks), np.float32)
        for c in range(NC):
            per_half = []
            for h in range(n_halves):
                lo = int(self.half_chunk_base[h]) * P
                hi = int(self.half_chunk_base[h + 1]) * P
                seg = idx_flat[c, lo:hi]
                wr = (np.tile(seg.reshape(-1, 16).T, (8, 1)) if hi > lo
                      else np.zeros((P, 0), np.int16))
                per_half.append(np.ascontiguousarray(wr))
            self.idx_wrapped.append(per_half)
            self.w_wrapped[c] = w_flat[c].reshape(total_chunks, P).T
            self.rel_wrapped[c] = rel_flat[c].reshape(total_chunks, P).T


# ---------------------------------------------------------------------------
# Device program pieces
# ---------------------------------------------------------------------------

def _dense_layer(nc, sb, psp, xT_in, n_node_tiles, wk0, wk1, b_row, ones, outb):
    """outb[:, m, :] = relu(xT[:, m-tile].T @ W + b) over both K halves."""
    xa0 = sb.tile([P, n_node_tiles * P], F32, tag="xa0")
    xa1 = sb.tile([P, n_node_tiles * P], F32, tag="xa1")
    nc.sync.dma_start(xa0[:], xT_in[0:P, :])
    nc.sync.dma_start(xa1[:], xT_in[P : 2 * P, :])
    for m in range(n_node_tiles):
        ps = psp.tile([P, P], F32, tag="psA")
        sl = slice(m * P, (m + 1) * P)
        nc.tensor.matmul(ps[:], lhsT=xa0[:, sl], rhs=wk0[:], start=True, stop=False)
        nc.tensor.matmul(ps[:], lhsT=xa1[:, sl], rhs=wk1[:], start=False, stop=False)
        nc.tensor.matmul(ps[:], lhsT=ones[:1, :], rhs=b_row[:1, :], start=False, stop=True)
        nc.scalar.activation(outb[:, m, :], ps[:], AF.Relu)


def _spmm(nc, gpool, spool, psp, plan, tables, idx_tiles, w_col, rel_col, iota,
          accN, accT, width, gtag, pstag):
    """Shared spmm walker. Per 128-edge chunk builds S = onehot(dst_rel)*w, then:
      accT given (Option T, width==P): psum[feat, dst] += G_chunk.T @ S
      accN given (Option N): psum[dst, 0:width] += S.T @ G_chunk -> accN[:, t, :]
    Halves accumulate into SBUF acc via copy-then-add."""
    n_tiles = plan.n_tiles
    written = [False] * n_tiles
    gchunk = 0
    for h in range(plan.n_halves):
        nch_h = plan.half_nchunks[h]
        if nch_h == 0:
            continue
        Lh = nch_h * P
        sched = plan.sched[h]
        table_ap, step = tables[h]
        ci = 0
        psum = None
        for b0 in range(0, Lh, IDX_BATCH):
            nidx = min(IDX_BATCH, Lh - b0)
            nch = nidx // P
            gt = gpool.tile([P, IDX_BATCH // P, width], F32, tag=gtag)
            if "gather" not in KNOCKOUT:
                nc.gpsimd.dma_gather(
                    gt[:, :nch, :], table_ap,
                    idx_tiles[h][:, b0 // 16 : (b0 + nidx) // 16],
                    nidx, nidx, width, elem_step=step, single_packet=SINGLE_PACKET,
                )
            else:
                nc.vector.memset(gt[:, :1, :8], 0.0)
            for k in range(nch):
                t, first, last = sched[ci]
                if first:
                    psum = psp.tile([P, width], F32, tag=pstag)
                S = spool.tile([P, P], F32, tag="S")
                if "dve" not in KNOCKOUT:
                    nc.vector.tensor_scalar(
                        S[:], iota[:], rel_col[:, gchunk : gchunk + 1],
                        w_col[:, gchunk : gchunk + 1], ALU.is_equal, ALU.mult,
                    )
                else:
                    nc.vector.memset(S[:, :8], 0.0)
                if "mm" not in KNOCKOUT:
                    if accN is not None:
                        nc.tensor.matmul(psum[:], lhsT=S[:], rhs=gt[:, k, :],
                                         start=first, stop=last)
                    else:
                        nc.tensor.matmul(psum[:], lhsT=gt[:, k, :], rhs=S[:],
                                         start=first, stop=last)
                elif first:
                    nc.tensor.matmul(psum[:, 0:P], lhsT=iota[:1, :], rhs=iota[:1, :],
                                     start=True, stop=True)
                if last:
                    sl = (accN[:, t, :] if accN is not None
                          else accT[:, t * P : (t + 1) * P])
                    if not written[t]:
                        nc.scalar.activation(sl, psum[:], AF.Copy)
                        written[t] = True
                    else:
                        nc.vector.tensor_tensor(sl, psum[:], sl, ALU.add)
                ci += 1
                gchunk += 1
    for t in range(n_tiles):
        if not written[t]:
            sl = accN[:, t, :] if accN is not None else accT[:, t * P : (t + 1) * P]
            nc.vector.memset(sl, 0.0)


def _gcn_second(nc, psp, accT, w_t, b_row, ones, outc, n_tiles):
    """outc[:, t, 0:O] = relu(accT_t.T @ W + b)."""
    for t in range(n_tiles):
        ps = psp.tile([P, P], F32, tag="ps2")
        nc.tensor.matmul(ps[:], lhsT=accT[:, t * P : (t + 1) * P], rhs=w_t[:],
                         start=True, stop=False)
        nc.tensor.matmul(ps[:], lhsT=ones[:1, :], rhs=b_row[:1, :], start=False, stop=True)
        nc.scalar.activation(outc[:, t, 0:O], ps[:], AF.Relu)


def _edge_phase_loads(nc, sb, plan, idx_in, w_in, rel_in, tagp):
    idx_t = []
    for h in range(plan.n_halves):
        it = sb.tile(list(plan.idx_wrapped[0][h].shape), I16, tag=f"{tagp}i{h}")
        nc.sync.dma_start(it[:], idx_in[h][:])
        idx_t.append(it)
    wct = sb.tile([P, plan.total_chunks], F32, tag=f"{tagp}w")
    relt = sb.tile([P, plan.total_chunks], F32, tag=f"{tagp}r")
    nc.sync.dma_start(wct[:], w_in[:])
    nc.sync.dma_start(relt[:], rel_in[:])
    return idx_t, wct, relt


# ---------------------------------------------------------------------------
# Full program
# ---------------------------------------------------------------------------

def build_program(p11, p22, p01, p02, repeat=1):
    nc = bacc.Bacc("TRN2", num_devices=NC, dynamic_dma_scratch_size=DMA_SCRATCH)

    x1T = nc.dram_tensor("x1T", [D, S1P], F32, kind="ExternalInput")
    x2T = nc.dram_tensor("x2T", [D, S2P], F32, kind="ExternalInput")
    wemb_sh = nc.dram_tensor("wemb_sh", [S2P, DW], F32, kind="ExternalInput")
    w1a = nc.dram_tensor("w1a", [D, O], F32, kind="ExternalInput")
    w1b = nc.dram_tensor("w1b", [O, O], F32, kind="ExternalInput")
    w2a = nc.dram_tensor("w2a", [D, O], F32, kind="ExternalInput")
    w2b = nc.dram_tensor("w2b", [O, O], F32, kind="ExternalInput")
    biases = nc.dram_tensor("biases", [4, O], F32, kind="ExternalInput")
    iota_in = nc.dram_tensor("iota", [P, P], F32, kind="ExternalInput")
    ones_in = nc.dram_tensor("ones", [1, P], F32, kind="ExternalInput")

    def edge_inputs(name, plan):
        idx = [
            nc.dram_tensor(f"{name}_idx{h}", list(plan.idx_wrapped[0][h].shape), I16,
                           kind="ExternalInput")
            for h in range(plan.n_halves)
        ]
        wv = nc.dram_tensor(f"{name}_w", [P, plan.total_chunks], F32, kind="ExternalInput")
        rel = nc.dram_tensor(f"{name}_rel", [P, plan.total_chunks], F32, kind="ExternalInput")
        return idx, wv, rel

    e11_in = edge_inputs("e11", p11)
    e22_in = edge_inputs("e22", p22)
    e01_in = edge_inputs("e01", p01)
    e02_in = edge_inputs("e02", p02)

    l11_loc = nc.dram_tensor("l11_loc", [S1P, O], F32)
    l12w_loc = nc.dram_tensor("l12w_loc", [S2P, 2 * O], F32)
    l21c_loc = nc.dram_tensor("l21c_loc", [S1P, 2 * O], F32)
    l2all_loc = nc.dram_tensor("l2all_loc", [S2P, 3 * O], F32)
    l11_full = nc.dram_tensor("l11_full", [NC * S1P, O], F32, addr_space="Shared")
    l12w_full = nc.dram_tensor("l12w_full", [NC * S2P, 2 * O], F32, addr_space="Shared")
    l21c_full = nc.dram_tensor("l21c_full", [NC * S1P, 2 * O], F32, addr_space="Shared")
    l2all_full = nc.dram_tensor("l2all_full", [NC * S2P, 3 * O], F32, addr_space="Shared")

    doc_loc = nc.dram_tensor("doc_local", [S0P, 2 * O + DW], F32, kind="ExternalOutput")
    docsvd_loc = nc.dram_tensor("docsvd_local", [S0P, 2 * O + DW], F32,
                                kind="ExternalOutput")

    rg = [list(range(NC))]

    def ag(inp, outp):
        if "ag" not in KNOCKOUT:
            nc.gpsimd.collective_compute(
                "AllGather", ALU.bypass, replica_groups=rg, ins=[inp[:]], outs=[outp[:]])

    def rearr(dram_ap):
        return dram_ap.rearrange("(t p) f -> p t f", p=P)

    with TileContext(nc) as tc:
        with tc.tile_pool(name="const", bufs=1) as cp:
            iota = cp.tile([P, P], F32)
            ones = cp.tile([1, P], F32)
            nc.sync.dma_start(iota[:], iota_in[:])
            nc.sync.dma_start(ones[:], ones_in[:])
            w1a0 = cp.tile([P, O], F32); nc.sync.dma_start(w1a0[:], w1a[0:P, :])
            w1a1 = cp.tile([P, O], F32); nc.sync.dma_start(w1a1[:], w1a[P:D, :])
            w2a0 = cp.tile([P, O], F32); nc.sync.dma_start(w2a0[:], w2a[0:P, :])
            w2a1 = cp.tile([P, O], F32); nc.sync.dma_start(w2a1[:], w2a[P:D, :])
            w1bt = cp.tile([O, O], F32); nc.sync.dma_start(w1bt[:], w1b[:])
            w2bt = cp.tile([O, O], F32); nc.sync.dma_start(w2bt[:], w2b[:])
            btiles = []
            for i in range(4):
                bt = cp.tile([1, O], F32, tag=f"b{i}")
                nc.sync.dma_start(bt[:], biases[i : i + 1, :])
                btiles.append(bt)
            b1a, b1b, b2a, b2b = (bt[:] for bt in btiles)

            def emit_body():
                with ExitStack() as ab:
                    psp = ab.enter_context(
                        tc.tile_pool(name="psum", bufs=2, space="PSUM"))
                    pspw = ab.enter_context(
                        tc.tile_pool(name="psumW", bufs=2, space="PSUM"))

                    # ---- phase A: identity GCN layers + wemb packing ----
                    with tc.tile_pool(name="phA", bufs=1) as sa:
                        outb1 = sa.tile([P, T1, O], F32)
                        _dense_layer(nc, sa, psp, x1T, T1, w1a0, w1a1, b1a, ones, outb1)
                        nc.sync.dma_start(rearr(l11_loc[:]), outb1[:])
                        outb2 = sa.tile([P, T2, O], F32)
                        _dense_layer(nc, sa, psp, x2T, T2, w2a0, w2a1, b2a, ones, outb2)
                        wt2 = sa.tile([P, T2, DW], F32)
                        nc.sync.dma_start(wt2[:], rearr(wemb_sh[:]))
                        nc.sync.dma_start(rearr(l12w_loc[:])[:, :, 0:O], outb2[:])
                        nc.sync.dma_start(rearr(l12w_loc[:])[:, :, O : 2 * O], wt2[:])

                    ag(l11_loc, l11_full)
                    ag(l12w_loc, l12w_full)

                    l11_halves = [(l11_full[0:HALF1, :], None),
                                  (l11_full[HALF1 : 2 * HALF1, :], None)]

                    # ---- phase B1: spmm(e11, l1_1) -> @W1b -> [l2_1 | l1_1] ----
                    with (
                        tc.tile_pool(name="phB1", bufs=1) as sb1,
                        tc.tile_pool(name="gpB1", bufs=GP_BUFS) as gp1,
                        tc.tile_pool(name="spB1", bufs=SP_BUFS) as sp1,
                    ):
                        idx_t, wct, relt = _edge_phase_loads(
                            nc, sb1, p11, e11_in[0], e11_in[1], e11_in[2], tagp="a")
                        accT = sb1.tile([P, S1P], F32, tag="accT")
                        _spmm(nc, gp1, sp1, psp, p11, l11_halves, idx_t, wct, relt,
                              iota, None, accT, P, "gT", "psT")
                        outc = sb1.tile([P, T1, 2 * O], F32, tag="outc")
                        _gcn_second(nc, pspw, accT, w1bt, b1b, ones, outc, T1)
                        nc.sync.dma_start(outc[:, :, O : 2 * O], rearr(l11_loc[:]))
                        nc.sync.dma_start(rearr(l21c_loc[:]), outc[:])

                    ag(l21c_loc, l21c_full)

                    # ---- phase B2: spmm(e22, l1_2) -> @W2b -> [l2_2 | l1_2 | wemb] ----
                    with (
                        tc.tile_pool(name="phB2", bufs=1) as sb2,
                        tc.tile_pool(name="gpB2", bufs=GP_BUFS) as gp2,
                        tc.tile_pool(name="spB2", bufs=SP_BUFS) as sp2,
                    ):
                        idx_t, wct, relt = _edge_phase_loads(
                            nc, sb2, p22, e22_in[0], e22_in[1], e22_in[2], tagp="b")
                        accT = sb2.tile([P, S2P], F32, tag="accT")
                        _spmm(nc, gp2, sp2, psp, p22, [(l12w_full[:, 0:O], 2 * O)],
                              idx_t, wct, relt, iota, None, accT, P, "gT", "psT")
                        outc2 = sb2.tile([P, T2, 3 * O], F32, tag="outc2")
                        _gcn_second(nc, pspw, accT, w2bt, b2b, ones, outc2, T2)
                        nc.sync.dma_start(outc2[:, :, O : 3 * O], rearr(l12w_loc[:]))
                        nc.sync.dma_start(rearr(l2all_loc[:]), outc2[:])

                    ag(l2all_loc, l2all_full)

                # ---- phase C: doc aggregation (A/B PSUM pools closed) ----
                with (
                    tc.tile_pool(name="phC", bufs=1) as sc,
                    tc.tile_pool(name="psC", bufs=2, space="PSUM") as psc,
                ):
                    acc01 = sc.tile([P, T0, 2 * O], F32, tag="acc01")  # [r0 | r0s]
                    acc02 = sc.tile([P, T0, 3 * O], F32, tag="acc02")  # [l22|l12|wemb]

                    with (
                        tc.tile_pool(name="gpC1", bufs=GP_BUFS) as gpc1,
                        tc.tile_pool(name="spC1", bufs=SP_BUFS) as spc1,
                    ):
                        idx_t, wct, relt = _edge_phase_loads(
                            nc, sc, p01, e01_in[0], e01_in[1], e01_in[2], tagp="c")
                        tables = [(l21c_full[0:HALF1, :], None),
                                  (l21c_full[HALF1 : 2 * HALF1, :], None)]
                        _spmm(nc, gpc1, spc1, psc, p01, tables, idx_t, wct, relt,
                              iota, acc01[:], None, 2 * O, "g01", "ps01")

                    with (
                        tc.tile_pool(name="gpC2", bufs=GP_BUFS) as gpc2,
                        tc.tile_pool(name="spC2", bufs=SP_BUFS) as spc2,
                    ):
                        idx_t, wct, relt = _edge_phase_loads(
                            nc, sc, p02, e02_in[0], e02_in[1], e02_in[2], tagp="d")
                        _spmm(nc, gpc2, spc2, psc, p02, [(l2all_full[:], None)],
                              idx_t, wct, relt, iota, acc02[:], None, 3 * O, "g02", "ps02")

                    docb = sc.tile([P, T0, 2 * O + DW], F32, tag="docb")
                    docsb = sc.tile([P, T0, 2 * O + DW], F32, tag="docsb")

                    def norm_scale(acc_slices, out_writes):
                        ss_total = None
                        for i, (s_ap, wdt) in enumerate(acc_slices):
                            sq = sc.tile([P, wdt], F32, tag=f"sq{i}")
                            ss = sc.tile([P, 1], F32, tag=f"ss{i}")
                            nc.scalar.activation(sq[:], s_ap, AF.Square, accum_out=ss[:])
                            if ss_total is None:
                                ss_total = ss
                            else:
                                nc.vector.tensor_tensor(
                                    ss_total[:], ss[:], ss_total[:], ALU.add)
                        nrm = sc.tile([P, 1], F32, tag="nrm")
                        nc.scalar.activation(nrm[:], ss_total[:], AF.Sqrt)
                        nc.vector.tensor_scalar_add(nrm[:], nrm[:], EPS)
                        rn = sc.tile([P, 1], F32, tag="rn")
                        nc.vector.reciprocal(rn[:], nrm[:])
                        for dst_ap, s_ap in out_writes:
                            nc.vector.tensor_scalar_mul(dst_ap, s_ap, rn[:])

                    for t in range(T0):
                        a01 = acc01[:, t, :]
                        a02 = acc02[:, t, :]
                        # doc = [norm(r0) | norm([l22|wemb])]
                        norm_scale([(a01[:, 0:O], O)],
                                   [(docb[:, t, 0:O], a01[:, 0:O])])
                        norm_scale(
                            [(a02[:, 0:O], O), (a02[:, 2 * O : 3 * O], O)],
                            [(docb[:, t, O : 2 * O], a02[:, 0:O]),
                             (docb[:, t, 2 * O : 3 * O], a02[:, 2 * O : 3 * O])])
                        # doc_svd = [norm(r0s) | norm([l12|wemb])]
                        norm_scale([(a01[:, O : 2 * O], O)],
                                   [(docsb[:, t, 0:O], a01[:, O : 2 * O])])
                        norm_scale([(a02[:, O : 3 * O], 2 * O)],
                                   [(docsb[:, t, O : 3 * O], a02[:, O : 3 * O])])
                    nc.sync.dma_start(rearr(doc_loc[:]), docb[:])
                    nc.sync.dma_start(rearr(docsvd_loc[:]), docsb[:])

            for _ in range(repeat):
                emit_body()

    nc.compile()
    return nc


# ---------------------------------------------------------------------------
# Host wrapper
# ---------------------------------------------------------------------------

_CACHE = {}


def _prep(inputs):
    x1 = np.asarray(inputs["x1"], np.float32)
    x2 = np.asarray(inputs["x2"], np.float32)
    wemb = np.asarray(inputs["word_emb"], np.float32)

    p11 = EdgePlan(inputs["e11_src"], inputs["e11_dst"], inputs["e11_w"],
                   S1, S1P, S1, S1P, HALF1)
    p22 = EdgePlan(inputs["e22_src"], inputs["e22_dst"], inputs["e22_w"],
                   S2, S2P, S2, S2P, None)
    p01 = EdgePlan(inputs["e01_src"], inputs["e01_dst"], inputs["e01_w"],
                   S0, S0P, S1, S1P, HALF1)
    p02 = EdgePlan(inputs["e02_src"], inputs["e02_dst"], inputs["e02_w"],
                   S0, S0P, S2, S2P, None)

    iota = np.tile(np.arange(P, dtype=np.float32), (P, 1))
    ones = np.ones((1, P), np.float32)
    biases = np.stack([
        np.asarray(inputs["b1a"], np.float32), np.asarray(inputs["b1b"], np.float32),
        np.asarray(inputs["b2a"], np.float32), np.asarray(inputs["b2b"], np.float32),
    ])

    in_maps = []
    for c in range(NC):
        x1T = np.zeros((D, S1P), np.float32)
        x1T[:, :S1] = x1[c * S1 : (c + 1) * S1].T
        x2T = np.zeros((D, S2P), np.float32)
        x2T[:, :S2] = x2[c * S2 : (c + 1) * S2].T
        wsh = np.zeros((S2P, DW), np.float32)
        wsh[:S2] = wemb[c * S2 : (c + 1) * S2]
        m = {
            "x1T": x1T, "x2T": x2T, "wemb_sh": wsh,
            "w1a": np.asarray(inputs["W1a"], np.float32),
            "w1b": np.asarray(inputs["W1b"], np.float32),
            "w2a": np.asarray(inputs["W2a"], np.float32),
            "w2b": np.asarray(inputs["W2b"], np.float32),
            "biases": biases, "iota": iota, "ones": ones,
        }
        for name, plan in (("e11", p11), ("e22", p22), ("e01", p01), ("e02", p02)):
            for h in range(plan.n_halves):
                m[f"{name}_idx{h}"] = plan.idx_wrapped[c][h]
            m[f"{name}_w"] = np.ascontiguousarray(plan.w_wrapped[c])
            m[f"{name}_rel"] = np.ascontiguousarray(plan.rel_wrapped[c])
        in_maps.append(m)
    return (p11, p22, p01, p02), in_maps


def get_compiled(inputs):
    plans, in_maps = _prep(inputs)
    key = tuple(p.total_chunks for p in plans) + tuple(
        tuple(p.n_chunks.reshape(-1).tolist()) for p in plans
    )
    if key not in _CACHE:
        _CACHE[key] = build_program(*plans)
    return _CACHE[key], in_maps


def kernel(**inputs):
    nc, in_maps = get_compiled(inputs)
    res = run_bass_kernel_spmd(nc, in_maps, core_ids=list(range(NC)), trace=False)
    doc = np.concatenate([res.results[c]["doc_local"][:S0] for c in range(NC)], axis=0)
    dsvd = np.concatenate([res.results[c]["docsvd_local"][:S0] for c in range(NC)],
                          axis=0)
    return (doc[:N0], dsvd[:N0])
